# revision 2
# baseline (speedup 1.0000x reference)
"""Trainium2 Bass kernel for the dual cross-attention transformer block (DAMT).

Strategy: pure data-parallel over batch — 8 samples, 8 NeuronCores, one sample
per core, no collectives. Per core the whole block (dual QKV projections,
bidirectional cross attention, output projection + residual + LayerNorm) runs
as one Tile program.

Dataflow avoids all on-chip transposes by computing in transposed layouts:
host ships G^T/T^T and pre-transposed weights; scores are computed as
s^T[k,q] = k·q with softmax along the PSUM partition axis (exp on ACT, column
sums via a ones-matrix matmul on the PE, normalization folded into the
PSUM->SBUF copy of the PV product).

Matmuls run in float32r (TF32-like, full PE rate at moving dim >= 256;
~1.5e-4 matmul rel err) except the PV/output-projection stage which runs in
bf16 to fit SBUF; accumulation is always fp32 in PSUM.

The program is specialized on input values that the reference harness holds
constant (zero biases, all-ones mask, identity LayerNorm); general fallback
paths are emitted when any of those are non-trivial.
"""
import math
import sys

sys.path.insert(0, "/opt/trn_rl_repo")

import numpy as np
import ml_dtypes

from concourse import bacc, bass, mybir
import concourse.tile as tile
from concourse.bass_utils import run_bass_kernel_spmd

F32 = mybir.dt.float32
F32R = mybir.dt.float32r
BF16 = mybir.dt.bfloat16
AF = mybir.ActivationFunctionType

B, S, H = 8, 1024, 1024
NH = 4
AH = 2 * H            # 2048, q/k inner size
DH = AH // NH         # 512, q/k head size
OUT = H               # 1024, v/out size
DV = OUT // NH        # 256, v head size
NKT = H // 128        # 8 contraction chunks
NST = S // 128        # 8 sequence tiles
SCALE = 1.0 / math.sqrt(DH)

_PROGRAM_CACHE = {}


def _bcast_row_ap(row_ap):
    """DRAM [1, N] row -> partition-broadcast [128, N] read AP for DMA."""
    return bass.AP(tensor=row_ap.tensor, offset=row_ap.offset,
                   ap=[[0, 128], list(row_ap.ap[-1])])


def _build_program(use_am, use_bqk, use_bfull, use_ln):
    nc = bacc.Bacc(None, target_bir_lowering=False)

    gt = nc.dram_tensor("gt", [H, S], F32R, kind="ExternalInput")
    tt = nc.dram_tensor("tt", [H, S], F32R, kind="ExternalInput")
    gn = nc.dram_tensor("gn", [S, H], F32, kind="ExternalInput")
    tn = nc.dram_tensor("tn", [S, H], F32, kind="ExternalInput")
    wq_g = nc.dram_tensor("wq_g", [H, AH], F32R, kind="ExternalInput")
    wk_g = nc.dram_tensor("wk_g", [H, AH], F32R, kind="ExternalInput")
    wv_g = nc.dram_tensor("wv_g", [H, OUT], F32R, kind="ExternalInput")
    wq_t = nc.dram_tensor("wq_t", [H, AH], F32R, kind="ExternalInput")
    wk_t = nc.dram_tensor("wk_t", [H, AH], F32R, kind="ExternalInput")
    wv_t = nc.dram_tensor("wv_t", [H, OUT], F32R, kind="ExternalInput")
    wo_g = nc.dram_tensor("wo_g", [OUT, H], BF16, kind="ExternalInput")
    wo_t = nc.dram_tensor("wo_t", [OUT, H], BF16, kind="ExternalInput")
    hg = nc.dram_tensor("hg", [S, H], F32, kind="ExternalOutput")
    ht = nc.dram_tensor("ht", [S, H], F32, kind="ExternalOutput")
    consts = None
    if use_am or use_bqk:
        consts = nc.dram_tensor("consts", [128, 72], F32, kind="ExternalInput")
    genvec = None
    if use_bfull or use_ln:
        genvec = nc.dram_tensor("genvec", [6, 1024], F32, kind="ExternalInput")

    with tile.TileContext(nc) as tc:
        with (
            tc.tile_pool(name="base", bufs=1) as base,
            tc.tile_pool(name="wpool", bufs=1) as wpool,
            tc.tile_pool(name="qkv", bufs=1) as qkv,
            tc.tile_pool(name="att", bufs=1) as att,
            tc.tile_pool(name="rp", bufs=2) as rp,
            tc.tile_pool(name="cx", bufs=1) as cx,
            tc.tile_pool(name="op", bufs=2) as op_,
            tc.tile_pool(name="stp", bufs=2) as stp,
            tc.tile_pool(name="ps_s", bufs=2, space="PSUM") as ps_s,
            tc.tile_pool(name="ps_sum", bufs=2, space="PSUM") as ps_sum,
            tc.tile_pool(name="ps_c", bufs=2, space="PSUM") as ps_c,
            tc.tile_pool(name="ps_mm", bufs=2, space="PSUM") as ps_mm,
        ):
            gt_sb = base.tile([128, NKT * S], F32R, name="gt_sb")
            tt_sb = base.tile([128, NKT * S], F32R, name="tt_sb")
            nc.sync.dma_start(out=gt_sb.rearrange("p (kt s) -> p kt s", kt=NKT),
                              in_=gt[:, :].rearrange("(kt p) s -> p kt s", p=128))
            nc.sync.dma_start(out=tt_sb.rearrange("p (kt s) -> p kt s", kt=NKT),
                              in_=tt[:, :].rearrange("(kt p) s -> p kt s", p=128))
            ones_bf = base.tile([128, 128], BF16, name="ones_bf")
            nc.vector.memset(ones_bf, 1.0)
            eps_sb = base.tile([128, 1], F32, name="eps_sb")
            nc.vector.memset(eps_sb, 1e-12)
            consts_sb = None
            if consts is not None:
                consts_sb = base.tile([128, 72], F32, name="consts_sb")
                nc.sync.dma_start(out=consts_sb, in_=consts[:, :])

            branches = [
                # (wq, wk, wv, wo, q-source, kv-source, residual, out, bq_col, bk_col,
                #  bfull_row, lnw_row, lnb_row)
                (wq_g, wk_g, wv_g, wo_g, gt_sb, tt_sb, gn, hg, 8, 24, 0, 2, 3),
                (wq_t, wk_t, wv_t, wo_t, tt_sb, gt_sb, tn, ht, 40, 56, 1, 4, 5),
            ]

            for (wqd, wkd, wvd, wod, src_q, src_kv, resid_d, out_d,
                 bq_col, bk_col, bfull_row, lnw_row, lnb_row) in branches:
                wo_sb = cx.tile([128, NKT * H], BF16, tag="wo", name="wo_sb")
                nc.sync.dma_start(
                    out=wo_sb.rearrange("p (kt hh) -> p kt hh", kt=NKT),
                    in_=wod[:, :].rearrange("(kt p) hh -> p kt hh", p=128))
                ctx_sb = cx.tile([128, NKT * S], BF16, tag="ctx", name="ctx_sb")

                for h in range(NH):
                    wq_sb = wpool.tile([128, NKT * DH], F32R, tag="wq", name="wq_sb")
                    nc.sync.dma_start(
                        out=wq_sb.rearrange("p (kt a) -> p kt a", kt=NKT),
                        in_=wqd[:, h * DH:(h + 1) * DH].rearrange("(kt p) a -> p kt a", p=128))
                    wk_sb = wpool.tile([128, NKT * DH], F32R, tag="wk", name="wk_sb")
                    nc.sync.dma_start(
                        out=wk_sb.rearrange("p (kt a) -> p kt a", kt=NKT),
                        in_=wkd[:, h * DH:(h + 1) * DH].rearrange("(kt p) a -> p kt a", p=128))
                    wv_sb = wpool.tile([128, NKT * DV], F32R, tag="wv", name="wv_sb")
                    nc.sync.dma_start(
                        out=wv_sb.rearrange("p (kt a) -> p kt a", kt=NKT),
                        in_=wvd[:, h * DV:(h + 1) * DV].rearrange("(kt p) a -> p kt a", p=128))

                    qT_sb = qkv.tile([128, 4 * S], F32R, tag="qT", name="qT_sb")
                    kT_sb = qkv.tile([128, 4 * S], F32R, tag="kT", name="kT_sb")
                    v_sb = qkv.tile([128, NST * DV], BF16, tag="v", name="v_sb")

                    # q / k projections: out[ah-chunk m, S-half n] (transposed layout)
                    for wsb, osb, bcol, src, on_act in (
                            (wq_sb, qT_sb, bq_col, src_q, True),
                            (wk_sb, kT_sb, bk_col, src_kv, False)):
                        for m in range(4):
                            for n in range(2):
                                pq = ps_mm.tile([128, 512], F32, tag="mm", name="pq")
                                for kt in range(NKT):
                                    nc.tensor.matmul(
                                        pq,
                                        lhsT=wsb[:, kt * DH + m * 128:kt * DH + (m + 1) * 128],
                                        rhs=src[:, kt * S + n * 512:kt * S + (n + 1) * 512],
                                        start=(kt == 0), stop=(kt == NKT - 1))
                                dst = osb[:, m * S + n * 512:m * S + (n + 1) * 512]
                                if use_bqk:
                                    col = bcol + h * 4 + m
                                    nc.scalar.activation(out=dst, in_=pq, func=AF.Identity,
                                                         bias=consts_sb[:, col:col + 1],
                                                         scale=1.0)
                                elif on_act:
                                    nc.scalar.copy(out=dst, in_=pq)
                                else:
                                    nc.vector.tensor_copy(out=dst, in_=pq)

                    # v projection: out[S-tile st, dv] natural layout, bf16
                    for st in range(NST):
                        pv = ps_mm.tile([128, 256], F32, tag="mm", name="pv")
                        for kt in range(NKT):
                            nc.tensor.matmul(
                                pv,
                                lhsT=src_kv[:, kt * S + st * 128:kt * S + st * 128 + 128],
                                rhs=wv_sb[:, kt * DV:(kt + 1) * DV],
                                start=(kt == 0), stop=(kt == NKT - 1))
                        nc.vector.tensor_copy(out=v_sb[:, st * DV:(st + 1) * DV], in_=pv)

                    # attention on two 512-wide query blocks
                    for blk in range(2):
                        pexp_sb = att.tile([128, NST * 512], BF16, tag="pexp", name="pexp_sb")
                        psum_sums = ps_sum.tile([128, 512], F32, tag="sums", name="psum_sums")
                        for j in range(NST):
                            pss = ps_s.tile([128, 512], F32, tag="sc", name="pss")
                            for m in range(4):
                                nc.tensor.matmul(
                                    pss,
                                    lhsT=kT_sb[:, m * S + j * 128:m * S + j * 128 + 128],
                                    rhs=qT_sb[:, m * S + blk * 512:m * S + (blk + 1) * 512],
                                    start=(m == 0), stop=(m == 3))
                            if use_am:
                                nc.scalar.activation(out=pexp_sb[:, j * 512:(j + 1) * 512],
                                                     in_=pss, func=AF.Exp,
                                                     bias=consts_sb[:, j:j + 1], scale=1.0)
                            else:
                                nc.scalar.activation(out=pexp_sb[:, j * 512:(j + 1) * 512],
                                                     in_=pss, func=AF.Exp)
                            nc.tensor.matmul(psum_sums, lhsT=ones_bf,
                                             rhs=pexp_sb[:, j * 512:(j + 1) * 512],
                                             start=(j == 0), stop=(j == NST - 1))
                        rinv = rp.tile([128, 512], F32, tag="rinv", name="rinv")
                        nc.vector.reciprocal(out=rinv, in_=psum_sums)
                        for dvh in range(2):
                            pc = ps_c.tile([128, 512], F32, tag="pc", name="pc")
                            for j in range(NST):
                                nc.tensor.matmul(
                                    pc,
                                    lhsT=v_sb[:, j * DV + dvh * 128:j * DV + dvh * 128 + 128],
                                    rhs=pexp_sb[:, j * 512:(j + 1) * 512],
                                    start=(j == 0), stop=(j == NST - 1))
                            crange = (h * 2 + dvh) * S + blk * 512
                            nc.vector.tensor_mul(out=ctx_sb[:, crange:crange + 512],
                                                 in0=pc, in1=rinv)

                # output projection + residual + LayerNorm
                lnw_sb = lnb_sb = bfull_sb = None
                if use_ln:
                    lnw_sb = base.tile([128, 1024], F32, tag="lnw", name="lnw_sb")
                    nc.sync.dma_start(out=lnw_sb, in_=_bcast_row_ap(genvec[lnw_row:lnw_row + 1, :]))
                    lnb_sb = base.tile([128, 1024], F32, tag="lnb", name="lnb_sb")
                    nc.sync.dma_start(out=lnb_sb, in_=_bcast_row_ap(genvec[lnb_row:lnb_row + 1, :]))
                if use_bfull:
                    bfull_sb = base.tile([128, 1024], F32, tag="bfull", name="bfull_sb")
                    nc.sync.dma_start(out=bfull_sb,
                                      in_=_bcast_row_ap(genvec[bfull_row:bfull_row + 1, :]))

                for st in range(NST):
                    resid_t = op_.tile([128, 1024], F32, tag="res", name="resid_t")
                    nc.sync.dma_start(out=resid_t, in_=resid_d[st * 128:(st + 1) * 128, :])
                    out_t = op_.tile([128, 1024], F32, tag="out", name="out_t")
                    for half in range(2):
                        po = ps_mm.tile([128, 512], F32, tag="mm", name="po")
                        for c in range(NKT):
                            nc.tensor.matmul(
                                po,
                                lhsT=ctx_sb[:, c * S + st * 128:c * S + st * 128 + 128],
                                rhs=wo_sb[:, c * H + half * 512:c * H + (half + 1) * 512],
                                start=(c == 0), stop=(c == NKT - 1))
                        nc.vector.tensor_add(out=out_t[:, half * 512:(half + 1) * 512],
                                             in0=po,
                                             in1=resid_t[:, half * 512:(half + 1) * 512])
                    if use_bfull:
                        nc.vector.tensor_add(out=out_t, in0=out_t, in1=bfull_sb)
                    # LayerNorm over the free (H) axis
                    stats = stp.tile([128, 2, 6], F32, tag="stats", name="stats")
                    for sg in range(2):
                        nc.vector.bn_stats(out=stats[:, sg, :],
                                           in_=out_t[:, sg * 512:(sg + 1) * 512])
                    mv = stp.tile([128, 2], F32, tag="mv", name="mv")
                    nc.vector.bn_aggr(out=mv, in_=stats)
                    rstd = stp.tile([128, 1], F32, tag="rstd", name="rstd")
                    nc.scalar.activation(out=rstd, in_=mv[:, 1:2], func=AF.Sqrt,
                                         bias=eps_sb, scale=1.0)
                    nc.vector.reciprocal(out=rstd, in_=rstd)
                    nc.vector.tensor_scalar(out=out_t, in0=out_t,
                                            scalar1=mv[:, 0:1], scalar2=rstd,
                                            op0=mybir.AluOpType.subtract,
                                            op1=mybir.AluOpType.mult)
                    if use_ln:
                        nc.vector.tensor_mul(out=out_t, in0=out_t, in1=lnw_sb)
                        nc.vector.tensor_add(out=out_t, in0=out_t, in1=lnb_sb)
                    nc.sync.dma_start(out=out_d[st * 128:(st + 1) * 128, :], in_=out_t)

    nc.finalize()
    return nc


def _get_program(flags):
    if flags not in _PROGRAM_CACHE:
        _PROGRAM_CACHE[flags] = _build_program(*flags)
    return _PROGRAM_CACHE[flags]


def prepare(G, T, mask, Wq, bq, WqT, bqT, Wk, bk, WkT, bkT, Wv, bv, WvT, bvT,
            Wg, bg, g_ln_w, g_ln_b, Wt, bt, t_ln_w, t_ln_b):
    """Host-side prep: flags, per-core input maps, and the built program."""
    f32 = np.float32
    G = np.asarray(G, f32)
    T = np.asarray(T, f32)
    mask = np.asarray(mask, f32)

    wq_g = np.ascontiguousarray((np.asarray(Wq, f32) * SCALE).T)
    wk_g = np.ascontiguousarray(np.asarray(Wk, f32).T)
    wv_g = np.ascontiguousarray(np.asarray(Wv, f32).T)
    wq_t = np.ascontiguousarray((np.asarray(WqT, f32) * SCALE).T)
    wk_t = np.ascontiguousarray(np.asarray(WkT, f32).T)
    wv_t = np.ascontiguousarray(np.asarray(WvT, f32).T)
    wo_g = np.ascontiguousarray(np.asarray(Wg, f32).T).astype(ml_dtypes.bfloat16)
    wo_t = np.ascontiguousarray(np.asarray(Wt, f32).T).astype(ml_dtypes.bfloat16)

    bq_eg = np.asarray(bq, f32) * SCALE
    bk_eg = np.asarray(bk, f32)
    bq_et = np.asarray(bqT, f32) * SCALE
    bk_et = np.asarray(bkT, f32)
    # ctx rows sum(p)=1, so the v bias passes through attention additively:
    # out += bv @ Wo.T + bo, folded into one post-projection vector.
    bfull_g = (np.asarray(bv, np.float64) @ np.asarray(Wg, np.float64).T
               + np.asarray(bg, np.float64)).astype(f32)
    bfull_t = (np.asarray(bvT, np.float64) @ np.asarray(Wt, np.float64).T
               + np.asarray(bt, np.float64)).astype(f32)
    lnw_g = np.asarray(g_ln_w, f32)
    lnb_g = np.asarray(g_ln_b, f32)
    lnw_t = np.asarray(t_ln_w, f32)
    lnb_t = np.asarray(t_ln_b, f32)

    use_am = not np.all(mask == 1.0)
    use_bqk = any(np.any(x != 0) for x in (bq_eg, bk_eg, bq_et, bk_et))
    use_bfull = bool(np.any(bfull_g != 0) or np.any(bfull_t != 0))
    use_ln = not (np.all(lnw_g == 1) and np.all(lnb_g == 0)
                  and np.all(lnw_t == 1) and np.all(lnb_t == 0))
    flags = (use_am, use_bqk, use_bfull, use_ln)
    nc = _get_program(flags)

    am_all = (1.0 - mask) * -10000.0  # [B, S]
    genvec = np.ascontiguousarray(
        np.stack([bfull_g, bfull_t, lnw_g, lnb_g, lnw_t, lnb_t]))

    in_maps = []
    for b in range(B):
        m = {
            "gt": np.ascontiguousarray(G[b].T),
            "tt": np.ascontiguousarray(T[b].T),
            "gn": np.ascontiguousarray(G[b]),
            "tn": np.ascontiguousarray(T[b]),
            "wq_g": wq_g, "wk_g": wk_g, "wv_g": wv_g,
            "wq_t": wq_t, "wk_t": wk_t, "wv_t": wv_t,
            "wo_g": wo_g, "wo_t": wo_t,
        }
        if use_am or use_bqk:
            consts = np.zeros((128, 72), f32)
            consts[:, 0:8] = am_all[b].reshape(8, 128).T
            consts[:, 8:24] = bq_eg.reshape(16, 128).T
            consts[:, 24:40] = bk_eg.reshape(16, 128).T
            consts[:, 40:56] = bq_et.reshape(16, 128).T
            consts[:, 56:72] = bk_et.reshape(16, 128).T
            m["consts"] = consts
        if use_bfull or use_ln:
            m["genvec"] = genvec
        in_maps.append(m)
    return nc, in_maps


def kernel(**inputs):
    nc, in_maps = prepare(**inputs)
    res = run_bass_kernel_spmd(nc, in_maps, core_ids=list(range(B)))
    H_G = np.stack([res.results[b]["hg"] for b in range(B)])
    H_T = np.stack([res.results[b]["ht"] for b in range(B)])
    return (H_G, H_T)


# revision 3
# speedup vs baseline: 167.1819x; 167.1819x over previous
"""Trainium2 Bass kernel for the dual cross-attention transformer block (DAMT).

Strategy: pure data-parallel over batch — 8 samples, 8 NeuronCores, one sample
per core, no collectives. Per core the whole block (dual QKV projections,
bidirectional cross attention, output projection + residual + LayerNorm) runs
as one Tile program.

Dataflow avoids all on-chip transposes by computing in transposed layouts:
host ships G^T/T^T and pre-transposed weights; scores are computed as
s^T[k,q] = k·q with softmax along the PSUM partition axis (exp on ACT, column
sums via a ones-matrix matmul on the PE, normalization folded into the
PSUM->SBUF copy of the PV product).

Matmuls run in float32r (TF32-like, full PE rate at moving dim >= 256;
~1.5e-4 matmul rel err) except the PV/output-projection stage which runs in
bf16 to fit SBUF; accumulation is always fp32 in PSUM.

The program is specialized on input values that the reference harness holds
constant (zero biases, all-ones mask, identity LayerNorm); general fallback
paths are emitted when any of those are non-trivial.
"""
import math
import sys

sys.path.insert(0, "/opt/trn_rl_repo")

import numpy as np
import ml_dtypes

from concourse import bacc, bass, mybir
import concourse.tile as tile
from concourse.bass_utils import run_bass_kernel_spmd

F32 = mybir.dt.float32
F32R = mybir.dt.float32r
BF16 = mybir.dt.bfloat16
AF = mybir.ActivationFunctionType

B, S, H = 8, 1024, 1024
NH = 4
AH = 2 * H            # 2048, q/k inner size
DH = AH // NH         # 512, q/k head size
OUT = H               # 1024, v/out size
DV = OUT // NH        # 256, v head size
NKT = H // 128        # 8 contraction chunks
NST = S // 128        # 8 sequence tiles
SCALE = 1.0 / math.sqrt(DH)

_PROGRAM_CACHE = {}


def _bcast_row_ap(row_ap):
    """DRAM [1, N] row -> partition-broadcast [128, N] read AP for DMA."""
    return bass.AP(tensor=row_ap.tensor, offset=row_ap.offset,
                   ap=[[0, 128], list(row_ap.ap[-1])])


def _build_program(use_am, use_bqk, use_bfull, use_ln, reps=1):
    nc = bacc.Bacc(None, target_bir_lowering=False)

    gt = nc.dram_tensor("gt", [H, S], F32R, kind="ExternalInput")
    tt = nc.dram_tensor("tt", [H, S], F32R, kind="ExternalInput")
    gn = nc.dram_tensor("gn", [S, H], F32, kind="ExternalInput")
    tn = nc.dram_tensor("tn", [S, H], F32, kind="ExternalInput")
    wq_g = nc.dram_tensor("wq_g", [H, AH], F32R, kind="ExternalInput")
    wk_g = nc.dram_tensor("wk_g", [H, AH], F32R, kind="ExternalInput")
    wv_g = nc.dram_tensor("wv_g", [H, OUT], F32R, kind="ExternalInput")
    wq_t = nc.dram_tensor("wq_t", [H, AH], F32R, kind="ExternalInput")
    wk_t = nc.dram_tensor("wk_t", [H, AH], F32R, kind="ExternalInput")
    wv_t = nc.dram_tensor("wv_t", [H, OUT], F32R, kind="ExternalInput")
    wo_g = nc.dram_tensor("wo_g", [OUT, H], BF16, kind="ExternalInput")
    wo_t = nc.dram_tensor("wo_t", [OUT, H], BF16, kind="ExternalInput")
    hg = nc.dram_tensor("hg", [S, H], F32, kind="ExternalOutput")
    ht = nc.dram_tensor("ht", [S, H], F32, kind="ExternalOutput")
    consts = None
    if use_am or use_bqk:
        consts = nc.dram_tensor("consts", [128, 72], F32, kind="ExternalInput")
    genvec = None
    if use_bfull or use_ln:
        genvec = nc.dram_tensor("genvec", [6, 1024], F32, kind="ExternalInput")

    with tile.TileContext(nc) as tc:
        with (
            tc.tile_pool(name="base", bufs=1) as base,
            tc.tile_pool(name="wpool", bufs=1) as wpool,
            tc.tile_pool(name="qkv", bufs=1) as qkv,
            tc.tile_pool(name="att", bufs=1) as att,
            tc.tile_pool(name="rp", bufs=2) as rp,
            tc.tile_pool(name="cx", bufs=1) as cx,
            tc.tile_pool(name="op", bufs=2) as op_,
            tc.tile_pool(name="stp", bufs=2) as stp,
            tc.tile_pool(name="ps_s", bufs=2, space="PSUM") as ps_s,
            tc.tile_pool(name="ps_sum", bufs=2, space="PSUM") as ps_sum,
            tc.tile_pool(name="ps_c", bufs=2, space="PSUM") as ps_c,
            tc.tile_pool(name="ps_mm", bufs=2, space="PSUM") as ps_mm,
        ):
            gt_sb = base.tile([128, NKT * S], F32R, name="gt_sb")
            tt_sb = base.tile([128, NKT * S], F32R, name="tt_sb")
            nc.sync.dma_start(out=gt_sb.rearrange("p (kt s) -> p kt s", kt=NKT),
                              in_=gt[:, :].rearrange("(kt p) s -> p kt s", p=128))
            nc.sync.dma_start(out=tt_sb.rearrange("p (kt s) -> p kt s", kt=NKT),
                              in_=tt[:, :].rearrange("(kt p) s -> p kt s", p=128))
            ones_bf = base.tile([128, 128], BF16, name="ones_bf")
            nc.vector.memset(ones_bf, 1.0)
            eps_sb = base.tile([128, 1], F32, name="eps_sb")
            nc.vector.memset(eps_sb, 1e-12)
            consts_sb = None
            if consts is not None:
                consts_sb = base.tile([128, 72], F32, name="consts_sb")
                nc.sync.dma_start(out=consts_sb, in_=consts[:, :])

            branches = [
                # (wq, wk, wv, wo, q-source, kv-source, residual, out, bq_col, bk_col,
                #  bfull_row, lnw_row, lnb_row)
                (wq_g, wk_g, wv_g, wo_g, gt_sb, tt_sb, gn, hg, 8, 24, 0, 2, 3),
                (wq_t, wk_t, wv_t, wo_t, tt_sb, gt_sb, tn, ht, 40, 56, 1, 4, 5),
            ] * reps

            for (wqd, wkd, wvd, wod, src_q, src_kv, resid_d, out_d,
                 bq_col, bk_col, bfull_row, lnw_row, lnb_row) in branches:
                wo_sb = cx.tile([128, NKT * H], BF16, tag="wo", name="wo_sb")
                nc.sync.dma_start(
                    out=wo_sb.rearrange("p (kt hh) -> p kt hh", kt=NKT),
                    in_=wod[:, :].rearrange("(kt p) hh -> p kt hh", p=128))
                ctx_sb = cx.tile([128, NKT * S], BF16, tag="ctx", name="ctx_sb")

                for h in range(NH):
                    wq_sb = wpool.tile([128, NKT * DH], F32R, tag="wq", name="wq_sb")
                    nc.sync.dma_start(
                        out=wq_sb.rearrange("p (kt a) -> p kt a", kt=NKT),
                        in_=wqd[:, h * DH:(h + 1) * DH].rearrange("(kt p) a -> p kt a", p=128))
                    wk_sb = wpool.tile([128, NKT * DH], F32R, tag="wk", name="wk_sb")
                    nc.sync.dma_start(
                        out=wk_sb.rearrange("p (kt a) -> p kt a", kt=NKT),
                        in_=wkd[:, h * DH:(h + 1) * DH].rearrange("(kt p) a -> p kt a", p=128))
                    wv_sb = wpool.tile([128, NKT * DV], F32R, tag="wv", name="wv_sb")
                    nc.sync.dma_start(
                        out=wv_sb.rearrange("p (kt a) -> p kt a", kt=NKT),
                        in_=wvd[:, h * DV:(h + 1) * DV].rearrange("(kt p) a -> p kt a", p=128))

                    qT_sb = qkv.tile([128, 4 * S], F32R, tag="qT", name="qT_sb")
                    kT_sb = qkv.tile([128, 4 * S], F32R, tag="kT", name="kT_sb")
                    v_sb = qkv.tile([128, NST * DV], BF16, tag="v", name="v_sb")

                    # q / k projections: out[ah-chunk m, S-half n] (transposed layout)
                    for wsb, osb, bcol, src, on_act in (
                            (wq_sb, qT_sb, bq_col, src_q, True),
                            (wk_sb, kT_sb, bk_col, src_kv, False)):
                        for m in range(4):
                            for n in range(2):
                                pq = ps_mm.tile([128, 512], F32, tag="mm", name="pq")
                                for kt in range(NKT):
                                    nc.tensor.matmul(
                                        pq,
                                        lhsT=wsb[:, kt * DH + m * 128:kt * DH + (m + 1) * 128],
                                        rhs=src[:, kt * S + n * 512:kt * S + (n + 1) * 512],
                                        start=(kt == 0), stop=(kt == NKT - 1))
                                dst = osb[:, m * S + n * 512:m * S + (n + 1) * 512]
                                if use_bqk:
                                    col = bcol + h * 4 + m
                                    nc.scalar.activation(out=dst, in_=pq, func=AF.Identity,
                                                         bias=consts_sb[:, col:col + 1],
                                                         scale=1.0)
                                elif on_act:
                                    nc.scalar.copy(out=dst, in_=pq)
                                else:
                                    nc.vector.tensor_copy(out=dst, in_=pq)

                    # v projection: out[S-tile st, dv] natural layout, bf16
                    for st in range(NST):
                        pv = ps_mm.tile([128, 256], F32, tag="mm", name="pv")
                        for kt in range(NKT):
                            nc.tensor.matmul(
                                pv,
                                lhsT=src_kv[:, kt * S + st * 128:kt * S + st * 128 + 128],
                                rhs=wv_sb[:, kt * DV:(kt + 1) * DV],
                                start=(kt == 0), stop=(kt == NKT - 1))
                        nc.vector.tensor_copy(out=v_sb[:, st * DV:(st + 1) * DV], in_=pv)

                    # attention on two 512-wide query blocks
                    for blk in range(2):
                        pexp_sb = att.tile([128, NST * 512], BF16, tag="pexp", name="pexp_sb")
                        psum_sums = ps_sum.tile([128, 512], F32, tag="sums", name="psum_sums")
                        for j in range(NST):
                            pss = ps_s.tile([128, 512], F32, tag="sc", name="pss")
                            for m in range(4):
                                nc.tensor.matmul(
                                    pss,
                                    lhsT=kT_sb[:, m * S + j * 128:m * S + j * 128 + 128],
                                    rhs=qT_sb[:, m * S + blk * 512:m * S + (blk + 1) * 512],
                                    start=(m == 0), stop=(m == 3))
                            if use_am:
                                nc.scalar.activation(out=pexp_sb[:, j * 512:(j + 1) * 512],
                                                     in_=pss, func=AF.Exp,
                                                     bias=consts_sb[:, j:j + 1], scale=1.0)
                            else:
                                nc.scalar.activation(out=pexp_sb[:, j * 512:(j + 1) * 512],
                                                     in_=pss, func=AF.Exp)
                            nc.tensor.matmul(psum_sums, lhsT=ones_bf,
                                             rhs=pexp_sb[:, j * 512:(j + 1) * 512],
                                             start=(j == 0), stop=(j == NST - 1))
                        rinv = rp.tile([128, 512], F32, tag="rinv", name="rinv")
                        nc.vector.reciprocal(out=rinv, in_=psum_sums)
                        for dvh in range(2):
                            pc = ps_c.tile([128, 512], F32, tag="pc", name="pc")
                            for j in range(NST):
                                nc.tensor.matmul(
                                    pc,
                                    lhsT=v_sb[:, j * DV + dvh * 128:j * DV + dvh * 128 + 128],
                                    rhs=pexp_sb[:, j * 512:(j + 1) * 512],
                                    start=(j == 0), stop=(j == NST - 1))
                            crange = (h * 2 + dvh) * S + blk * 512
                            nc.vector.tensor_mul(out=ctx_sb[:, crange:crange + 512],
                                                 in0=pc, in1=rinv)

                # output projection + residual + LayerNorm
                lnw_sb = lnb_sb = bfull_sb = None
                if use_ln:
                    lnw_sb = base.tile([128, 1024], F32, tag="lnw", name="lnw_sb")
                    nc.sync.dma_start(out=lnw_sb, in_=_bcast_row_ap(genvec[lnw_row:lnw_row + 1, :]))
                    lnb_sb = base.tile([128, 1024], F32, tag="lnb", name="lnb_sb")
                    nc.sync.dma_start(out=lnb_sb, in_=_bcast_row_ap(genvec[lnb_row:lnb_row + 1, :]))
                if use_bfull:
                    bfull_sb = base.tile([128, 1024], F32, tag="bfull", name="bfull_sb")
                    nc.sync.dma_start(out=bfull_sb,
                                      in_=_bcast_row_ap(genvec[bfull_row:bfull_row + 1, :]))

                for st in range(NST):
                    resid_t = op_.tile([128, 1024], F32, tag="res", name="resid_t")
                    nc.sync.dma_start(out=resid_t, in_=resid_d[st * 128:(st + 1) * 128, :])
                    out_t = op_.tile([128, 1024], F32, tag="out", name="out_t")
                    for half in range(2):
                        po = ps_mm.tile([128, 512], F32, tag="mm", name="po")
                        for c in range(NKT):
                            nc.tensor.matmul(
                                po,
                                lhsT=ctx_sb[:, c * S + st * 128:c * S + st * 128 + 128],
                                rhs=wo_sb[:, c * H + half * 512:c * H + (half + 1) * 512],
                                start=(c == 0), stop=(c == NKT - 1))
                        nc.vector.tensor_add(out=out_t[:, half * 512:(half + 1) * 512],
                                             in0=po,
                                             in1=resid_t[:, half * 512:(half + 1) * 512])
                    if use_bfull:
                        nc.vector.tensor_add(out=out_t, in0=out_t, in1=bfull_sb)
                    # LayerNorm over the free (H) axis
                    stats = stp.tile([128, 2, 6], F32, tag="stats", name="stats")
                    for sg in range(2):
                        nc.vector.bn_stats(out=stats[:, sg, :],
                                           in_=out_t[:, sg * 512:(sg + 1) * 512])
                    mv = stp.tile([128, 2], F32, tag="mv", name="mv")
                    nc.vector.bn_aggr(out=mv, in_=stats)
                    rstd = stp.tile([128, 1], F32, tag="rstd", name="rstd")
                    nc.scalar.activation(out=rstd, in_=mv[:, 1:2], func=AF.Sqrt,
                                         bias=eps_sb, scale=1.0)
                    nc.vector.reciprocal(out=rstd, in_=rstd)
                    nc.vector.tensor_scalar(out=out_t, in0=out_t,
                                            scalar1=mv[:, 0:1], scalar2=rstd,
                                            op0=mybir.AluOpType.subtract,
                                            op1=mybir.AluOpType.mult)
                    if use_ln:
                        nc.vector.tensor_mul(out=out_t, in0=out_t, in1=lnw_sb)
                        nc.vector.tensor_add(out=out_t, in0=out_t, in1=lnb_sb)
                    nc.sync.dma_start(out=out_d[st * 128:(st + 1) * 128, :], in_=out_t)

    nc.finalize()
    return nc


def _get_program(flags):
    if flags not in _PROGRAM_CACHE:
        _PROGRAM_CACHE[flags] = _build_program(*flags)
    return _PROGRAM_CACHE[flags]


def prepare(G, T, mask, Wq, bq, WqT, bqT, Wk, bk, WkT, bkT, Wv, bv, WvT, bvT,
            Wg, bg, g_ln_w, g_ln_b, Wt, bt, t_ln_w, t_ln_b):
    """Host-side prep: flags, per-core input maps, and the built program."""
    f32 = np.float32
    G = np.asarray(G, f32)
    T = np.asarray(T, f32)
    mask = np.asarray(mask, f32)

    wq_g = np.ascontiguousarray((np.asarray(Wq, f32) * SCALE).T)
    wk_g = np.ascontiguousarray(np.asarray(Wk, f32).T)
    wv_g = np.ascontiguousarray(np.asarray(Wv, f32).T)
    wq_t = np.ascontiguousarray((np.asarray(WqT, f32) * SCALE).T)
    wk_t = np.ascontiguousarray(np.asarray(WkT, f32).T)
    wv_t = np.ascontiguousarray(np.asarray(WvT, f32).T)
    wo_g = np.ascontiguousarray(np.asarray(Wg, f32).T).astype(ml_dtypes.bfloat16)
    wo_t = np.ascontiguousarray(np.asarray(Wt, f32).T).astype(ml_dtypes.bfloat16)

    bq_eg = np.asarray(bq, f32) * SCALE
    bk_eg = np.asarray(bk, f32)
    bq_et = np.asarray(bqT, f32) * SCALE
    bk_et = np.asarray(bkT, f32)
    # ctx rows sum(p)=1, so the v bias passes through attention additively:
    # out += bv @ Wo.T + bo, folded into one post-projection vector.
    bfull_g = (np.asarray(bv, np.float64) @ np.asarray(Wg, np.float64).T
               + np.asarray(bg, np.float64)).astype(f32)
    bfull_t = (np.asarray(bvT, np.float64) @ np.asarray(Wt, np.float64).T
               + np.asarray(bt, np.float64)).astype(f32)
    lnw_g = np.asarray(g_ln_w, f32)
    lnb_g = np.asarray(g_ln_b, f32)
    lnw_t = np.asarray(t_ln_w, f32)
    lnb_t = np.asarray(t_ln_b, f32)

    use_am = not np.all(mask == 1.0)
    use_bqk = any(np.any(x != 0) for x in (bq_eg, bk_eg, bq_et, bk_et))
    use_bfull = bool(np.any(bfull_g != 0) or np.any(bfull_t != 0))
    use_ln = not (np.all(lnw_g == 1) and np.all(lnb_g == 0)
                  and np.all(lnw_t == 1) and np.all(lnb_t == 0))
    flags = (use_am, use_bqk, use_bfull, use_ln)
    nc = _get_program(flags)

    am_all = (1.0 - mask) * -10000.0  # [B, S]
    genvec = np.ascontiguousarray(
        np.stack([bfull_g, bfull_t, lnw_g, lnb_g, lnw_t, lnb_t]))

    in_maps = []
    for b in range(B):
        m = {
            "gt": np.ascontiguousarray(G[b].T),
            "tt": np.ascontiguousarray(T[b].T),
            "gn": np.ascontiguousarray(G[b]),
            "tn": np.ascontiguousarray(T[b]),
            "wq_g": wq_g, "wk_g": wk_g, "wv_g": wv_g,
            "wq_t": wq_t, "wk_t": wk_t, "wv_t": wv_t,
            "wo_g": wo_g, "wo_t": wo_t,
        }
        if use_am or use_bqk:
            consts = np.zeros((128, 72), f32)
            consts[:, 0:8] = am_all[b].reshape(8, 128).T
            consts[:, 8:24] = bq_eg.reshape(16, 128).T
            consts[:, 24:40] = bk_eg.reshape(16, 128).T
            consts[:, 40:56] = bq_et.reshape(16, 128).T
            consts[:, 56:72] = bk_et.reshape(16, 128).T
            m["consts"] = consts
        if use_bfull or use_ln:
            m["genvec"] = genvec
        in_maps.append(m)
    return nc, in_maps


def kernel(**inputs):
    nc, in_maps = prepare(**inputs)
    res = run_bass_kernel_spmd(nc, in_maps, core_ids=list(range(B)))
    H_G = np.stack([res.results[b]["hg"] for b in range(B)])
    H_T = np.stack([res.results[b]["ht"] for b in range(B)])
    return (H_G, H_T)


# revision 6
# speedup vs baseline: 280.9228x; 1.6803x over previous
"""Trainium2 Bass kernel for the dual cross-attention transformer block (DAMT).

Strategy: pure data-parallel over batch — 8 samples, 8 NeuronCores, one sample
per core, no collectives. Per core the whole block (dual QKV projections,
bidirectional cross attention, output projection + residual + LayerNorm) runs
as one Tile program.

Dataflow avoids all on-chip transposes by computing in transposed layouts:
host ships G^T/T^T and pre-transposed weights; scores are computed as
s^T[k,q] = k·q with softmax along the PSUM partition axis (exp on ACT, column
sums via a ones-matrix matmul on the PE, normalization folded into the
PSUM->SBUF copy of the PV product).

Matmuls run in float32r (TF32-like, full PE rate at moving dim >= 256;
~1.5e-4 matmul rel err) except the PV/output-projection stage which runs in
bf16 to fit SBUF; accumulation is always fp32 in PSUM.

The program is specialized on input values that the reference harness holds
constant (zero biases, all-ones mask, identity LayerNorm); general fallback
paths are emitted when any of those are non-trivial.
"""
import math
import sys

sys.path.insert(0, "/opt/trn_rl_repo")

import numpy as np
import ml_dtypes

from concourse import bacc, bass, mybir
import concourse.tile as tile
from concourse.bass_utils import run_bass_kernel_spmd

F32 = mybir.dt.float32
F32R = mybir.dt.float32r
BF16 = mybir.dt.bfloat16
AF = mybir.ActivationFunctionType

B, S, H = 8, 1024, 1024
NH = 4
AH = 2 * H            # 2048, q/k inner size
DH = AH // NH         # 512, q/k head size
OUT = H               # 1024, v/out size
DV = OUT // NH        # 256, v head size
NKT = H // 128        # 8 contraction chunks
NST = S // 128        # 8 sequence tiles
SCALE = 1.0 / math.sqrt(DH)

_PROGRAM_CACHE = {}


def _bcast_row_ap(row_ap):
    """DRAM [1, N] row -> partition-broadcast [128, N] read AP for DMA."""
    return bass.AP(tensor=row_ap.tensor, offset=row_ap.offset,
                   ap=[[0, 128], list(row_ap.ap[-1])])


def _build_program(use_am, use_bqk, use_bfull, use_ln, reps=1):
    nc = bacc.Bacc(None, target_bir_lowering=False)

    gt = nc.dram_tensor("gt", [H, S], F32R, kind="ExternalInput")
    tt = nc.dram_tensor("tt", [H, S], F32R, kind="ExternalInput")
    gn = nc.dram_tensor("gn", [S, H], F32, kind="ExternalInput")
    tn = nc.dram_tensor("tn", [S, H], F32, kind="ExternalInput")
    wq_g = nc.dram_tensor("wq_g", [H, AH], F32R, kind="ExternalInput")
    wk_g = nc.dram_tensor("wk_g", [H, AH], F32R, kind="ExternalInput")
    wv_g = nc.dram_tensor("wv_g", [H, OUT], F32R, kind="ExternalInput")
    wq_t = nc.dram_tensor("wq_t", [H, AH], F32R, kind="ExternalInput")
    wk_t = nc.dram_tensor("wk_t", [H, AH], F32R, kind="ExternalInput")
    wv_t = nc.dram_tensor("wv_t", [H, OUT], F32R, kind="ExternalInput")
    wo_g = nc.dram_tensor("wo_g", [OUT, H], BF16, kind="ExternalInput")
    wo_t = nc.dram_tensor("wo_t", [OUT, H], BF16, kind="ExternalInput")
    hg = nc.dram_tensor("hg", [S, H], F32, kind="ExternalOutput")
    ht = nc.dram_tensor("ht", [S, H], F32, kind="ExternalOutput")
    consts = None
    if use_am or use_bqk:
        consts = nc.dram_tensor("consts", [128, 72], F32, kind="ExternalInput")
    genvec = None
    if use_bfull or use_ln:
        genvec = nc.dram_tensor("genvec", [6, 1024], F32, kind="ExternalInput")

    # The general path (ln/bfull broadcast tiles) needs ~12KB/partition more
    # SBUF; fund it by dropping double-buffering on the epilogue pools.
    ep_bufs = 1 if (use_bfull or use_ln) else 2

    with tile.TileContext(nc) as tc:
        with (
            tc.tile_pool(name="base", bufs=1) as base,
            tc.tile_pool(name="wpool", bufs=1) as wpool,
            tc.tile_pool(name="qkv", bufs=1) as qkv,
            tc.tile_pool(name="att", bufs=1) as att,
            tc.tile_pool(name="rp", bufs=ep_bufs) as rp,
            tc.tile_pool(name="cx", bufs=1) as cx,
            tc.tile_pool(name="op", bufs=ep_bufs) as op_,
            tc.tile_pool(name="stp", bufs=ep_bufs) as stp,
            tc.tile_pool(name="ps_s", bufs=2, space="PSUM") as ps_s,
            tc.tile_pool(name="ps_sum", bufs=2, space="PSUM") as ps_sum,
            tc.tile_pool(name="ps_c", bufs=2, space="PSUM") as ps_c,
            tc.tile_pool(name="ps_mm", bufs=2, space="PSUM") as ps_mm,
        ):
            gt_sb = base.tile([128, NKT * S], F32R, name="gt_sb")
            tt_sb = base.tile([128, NKT * S], F32R, name="tt_sb")
            # chunked loads so the first projections start before the full
            # input transfer completes
            for kt in range(NKT):
                nc.sync.dma_start(out=gt_sb[:, kt * S:(kt + 1) * S],
                                  in_=gt[kt * 128:(kt + 1) * 128, :])
                nc.sync.dma_start(out=tt_sb[:, kt * S:(kt + 1) * S],
                                  in_=tt[kt * 128:(kt + 1) * 128, :])
            ones_bf = base.tile([128, 128], BF16, name="ones_bf")
            nc.vector.memset(ones_bf, 1.0)
            eps_sb = base.tile([128, 1], F32, name="eps_sb")
            nc.vector.memset(eps_sb, 1e-12)
            consts_sb = None
            if consts is not None:
                consts_sb = base.tile([128, 72], F32, name="consts_sb")
                nc.sync.dma_start(out=consts_sb, in_=consts[:, :])

            branches = [
                # (wq, wk, wv, wo, q-source, kv-source, residual, out, bq_col, bk_col,
                #  bfull_row, lnw_row, lnb_row)
                (wq_g, wk_g, wv_g, wo_g, gt_sb, tt_sb, gn, hg, 8, 24, 0, 2, 3),
                (wq_t, wk_t, wv_t, wo_t, tt_sb, gt_sb, tn, ht, 40, 56, 1, 4, 5),
            ] * reps

            for (wqd, wkd, wvd, wod, src_q, src_kv, resid_d, out_d,
                 bq_col, bk_col, bfull_row, lnw_row, lnb_row) in branches:
                wo_sb = cx.tile([128, NKT * H], BF16, tag="wo", name="wo_sb")
                nc.sync.dma_start(
                    out=wo_sb.rearrange("p (kt hh) -> p kt hh", kt=NKT),
                    in_=wod[:, :].rearrange("(kt p) hh -> p kt hh", p=128))
                ctx_sb = cx.tile([128, NKT * S], BF16, tag="ctx", name="ctx_sb")

                for h in range(NH):
                    wq_sb = wpool.tile([128, NKT * DH], F32R, tag="wq", name="wq_sb")
                    nc.sync.dma_start(
                        out=wq_sb.rearrange("p (kt a) -> p kt a", kt=NKT),
                        in_=wqd[:, h * DH:(h + 1) * DH].rearrange("(kt p) a -> p kt a", p=128))
                    wk_sb = wpool.tile([128, NKT * DH], F32R, tag="wk", name="wk_sb")
                    nc.sync.dma_start(
                        out=wk_sb.rearrange("p (kt a) -> p kt a", kt=NKT),
                        in_=wkd[:, h * DH:(h + 1) * DH].rearrange("(kt p) a -> p kt a", p=128))
                    wv_sb = wpool.tile([128, NKT * DV], F32R, tag="wv", name="wv_sb")
                    nc.sync.dma_start(
                        out=wv_sb.rearrange("p (kt a) -> p kt a", kt=NKT),
                        in_=wvd[:, h * DV:(h + 1) * DV].rearrange("(kt p) a -> p kt a", p=128))

                    qT_sb = qkv.tile([128, 4 * S], F32R, tag="qT", name="qT_sb")
                    kT_sb = qkv.tile([128, 4 * S], F32R, tag="kT", name="kT_sb")
                    v_sb = qkv.tile([128, NST * DV], BF16, tag="v", name="v_sb")

                    # q / k projections: out[ah-chunk m, S-half n] (transposed layout)
                    for wsb, osb, bcol, src, on_act in (
                            (wq_sb, qT_sb, bq_col, src_q, True),
                            (wk_sb, kT_sb, bk_col, src_kv, False)):
                        for m in range(4):
                            for n in range(2):
                                pq = ps_mm.tile([128, 512], F32, tag="mm", name="pq")
                                for kt in range(NKT):
                                    nc.tensor.matmul(
                                        pq,
                                        lhsT=wsb[:, kt * DH + m * 128:kt * DH + (m + 1) * 128],
                                        rhs=src[:, kt * S + n * 512:kt * S + (n + 1) * 512],
                                        start=(kt == 0), stop=(kt == NKT - 1))
                                dst = osb[:, m * S + n * 512:m * S + (n + 1) * 512]
                                if use_bqk:
                                    col = bcol + h * 4 + m
                                    nc.scalar.activation(out=dst, in_=pq, func=AF.Identity,
                                                         bias=consts_sb[:, col:col + 1],
                                                         scale=1.0)
                                elif on_act:
                                    nc.scalar.copy(out=dst, in_=pq)
                                else:
                                    nc.vector.tensor_copy(out=dst, in_=pq)

                    # v projection: out[S-tile st, dv] natural layout, bf16
                    for st in range(NST):
                        pv = ps_mm.tile([128, 256], F32, tag="mm", name="pv")
                        for kt in range(NKT):
                            nc.tensor.matmul(
                                pv,
                                lhsT=src_kv[:, kt * S + st * 128:kt * S + st * 128 + 128],
                                rhs=wv_sb[:, kt * DV:(kt + 1) * DV],
                                start=(kt == 0), stop=(kt == NKT - 1))
                        nc.vector.tensor_copy(out=v_sb[:, st * DV:(st + 1) * DV], in_=pv)

                    # attention on two 512-wide query blocks
                    for blk in range(2):
                        pexp_sb = att.tile([128, NST * 512], BF16, tag="pexp", name="pexp_sb")
                        psum_sums = ps_sum.tile([128, 512], F32, tag="sums", name="psum_sums")
                        for j in range(NST):
                            pss = ps_s.tile([128, 512], F32, tag="sc", name="pss")
                            for m in range(4):
                                nc.tensor.matmul(
                                    pss,
                                    lhsT=kT_sb[:, m * S + j * 128:m * S + j * 128 + 128],
                                    rhs=qT_sb[:, m * S + blk * 512:m * S + (blk + 1) * 512],
                                    start=(m == 0), stop=(m == 3))
                            if use_am:
                                nc.scalar.activation(out=pexp_sb[:, j * 512:(j + 1) * 512],
                                                     in_=pss, func=AF.Exp,
                                                     bias=consts_sb[:, j:j + 1], scale=1.0)
                            else:
                                nc.scalar.activation(out=pexp_sb[:, j * 512:(j + 1) * 512],
                                                     in_=pss, func=AF.Exp)
                            nc.tensor.matmul(psum_sums, lhsT=ones_bf,
                                             rhs=pexp_sb[:, j * 512:(j + 1) * 512],
                                             start=(j == 0), stop=(j == NST - 1))
                        rinv = rp.tile([128, 512], F32, tag="rinv", name="rinv")
                        nc.vector.reciprocal(out=rinv, in_=psum_sums)
                        for dvh in range(2):
                            pc = ps_c.tile([128, 512], F32, tag="pc", name="pc")
                            for j in range(NST):
                                nc.tensor.matmul(
                                    pc,
                                    lhsT=v_sb[:, j * DV + dvh * 128:j * DV + dvh * 128 + 128],
                                    rhs=pexp_sb[:, j * 512:(j + 1) * 512],
                                    start=(j == 0), stop=(j == NST - 1))
                            crange = (h * 2 + dvh) * S + blk * 512
                            nc.vector.tensor_mul(out=ctx_sb[:, crange:crange + 512],
                                                 in0=pc, in1=rinv)

                # output projection + residual + LayerNorm
                lnw_sb = lnb_sb = bfull_sb = None
                if use_ln:
                    lnw_sb = base.tile([128, 1024], F32, tag="lnw", name="lnw_sb")
                    nc.sync.dma_start(out=lnw_sb, in_=_bcast_row_ap(genvec[lnw_row:lnw_row + 1, :]))
                    lnb_sb = base.tile([128, 1024], F32, tag="lnb", name="lnb_sb")
                    nc.sync.dma_start(out=lnb_sb, in_=_bcast_row_ap(genvec[lnb_row:lnb_row + 1, :]))
                if use_bfull:
                    bfull_sb = base.tile([128, 1024], F32, tag="bfull", name="bfull_sb")
                    nc.sync.dma_start(out=bfull_sb,
                                      in_=_bcast_row_ap(genvec[bfull_row:bfull_row + 1, :]))

                for st in range(NST):
                    resid_t = op_.tile([128, 1024], F32, tag="res", name="resid_t")
                    nc.sync.dma_start(out=resid_t, in_=resid_d[st * 128:(st + 1) * 128, :])
                    out_t = op_.tile([128, 1024], F32, tag="out", name="out_t")
                    for half in range(2):
                        po = ps_mm.tile([128, 512], F32, tag="mm", name="po")
                        for c in range(NKT):
                            nc.tensor.matmul(
                                po,
                                lhsT=ctx_sb[:, c * S + st * 128:c * S + st * 128 + 128],
                                rhs=wo_sb[:, c * H + half * 512:c * H + (half + 1) * 512],
                                start=(c == 0), stop=(c == NKT - 1))
                        nc.vector.tensor_add(out=out_t[:, half * 512:(half + 1) * 512],
                                             in0=po,
                                             in1=resid_t[:, half * 512:(half + 1) * 512])
                    if use_bfull:
                        nc.vector.tensor_add(out=out_t, in0=out_t, in1=bfull_sb)
                    # LayerNorm over the free (H) axis
                    stats = stp.tile([128, 2, 6], F32, tag="stats", name="stats")
                    for sg in range(2):
                        nc.vector.bn_stats(out=stats[:, sg, :],
                                           in_=out_t[:, sg * 512:(sg + 1) * 512])
                    mv = stp.tile([128, 2], F32, tag="mv", name="mv")
                    nc.vector.bn_aggr(out=mv, in_=stats)
                    rstd = stp.tile([128, 1], F32, tag="rstd", name="rstd")
                    nc.scalar.activation(out=rstd, in_=mv[:, 1:2], func=AF.Sqrt,
                                         bias=eps_sb, scale=1.0)
                    nc.vector.reciprocal(out=rstd, in_=rstd)
                    nc.vector.tensor_scalar(out=out_t, in0=out_t,
                                            scalar1=mv[:, 0:1], scalar2=rstd,
                                            op0=mybir.AluOpType.subtract,
                                            op1=mybir.AluOpType.mult)
                    if use_ln:
                        nc.vector.tensor_mul(out=out_t, in0=out_t, in1=lnw_sb)
                        nc.vector.tensor_add(out=out_t, in0=out_t, in1=lnb_sb)
                    nc.sync.dma_start(out=out_d[st * 128:(st + 1) * 128, :], in_=out_t)

    nc.finalize()
    return nc


def _get_program(flags):
    if flags not in _PROGRAM_CACHE:
        _PROGRAM_CACHE[flags] = _build_program(*flags)
    return _PROGRAM_CACHE[flags]


def prepare(G, T, mask, Wq, bq, WqT, bqT, Wk, bk, WkT, bkT, Wv, bv, WvT, bvT,
            Wg, bg, g_ln_w, g_ln_b, Wt, bt, t_ln_w, t_ln_b):
    """Host-side prep: flags, per-core input maps, and the built program."""
    f32 = np.float32
    G = np.asarray(G, f32)
    T = np.asarray(T, f32)
    mask = np.asarray(mask, f32)

    wq_g = np.ascontiguousarray((np.asarray(Wq, f32) * SCALE).T)
    wk_g = np.ascontiguousarray(np.asarray(Wk, f32).T)
    wv_g = np.ascontiguousarray(np.asarray(Wv, f32).T)
    wq_t = np.ascontiguousarray((np.asarray(WqT, f32) * SCALE).T)
    wk_t = np.ascontiguousarray(np.asarray(WkT, f32).T)
    wv_t = np.ascontiguousarray(np.asarray(WvT, f32).T)
    wo_g = np.ascontiguousarray(np.asarray(Wg, f32).T).astype(ml_dtypes.bfloat16)
    wo_t = np.ascontiguousarray(np.asarray(Wt, f32).T).astype(ml_dtypes.bfloat16)

    bq_eg = np.asarray(bq, f32) * SCALE
    bk_eg = np.asarray(bk, f32)
    bq_et = np.asarray(bqT, f32) * SCALE
    bk_et = np.asarray(bkT, f32)
    # ctx rows sum(p)=1, so the v bias passes through attention additively:
    # out += bv @ Wo.T + bo, folded into one post-projection vector.
    bfull_g = (np.asarray(bv, np.float64) @ np.asarray(Wg, np.float64).T
               + np.asarray(bg, np.float64)).astype(f32)
    bfull_t = (np.asarray(bvT, np.float64) @ np.asarray(Wt, np.float64).T
               + np.asarray(bt, np.float64)).astype(f32)
    lnw_g = np.asarray(g_ln_w, f32)
    lnb_g = np.asarray(g_ln_b, f32)
    lnw_t = np.asarray(t_ln_w, f32)
    lnb_t = np.asarray(t_ln_b, f32)

    use_am = not np.all(mask == 1.0)
    use_bqk = any(np.any(x != 0) for x in (bq_eg, bk_eg, bq_et, bk_et))
    use_bfull = bool(np.any(bfull_g != 0) or np.any(bfull_t != 0))
    use_ln = not (np.all(lnw_g == 1) and np.all(lnb_g == 0)
                  and np.all(lnw_t == 1) and np.all(lnb_t == 0))
    flags = (use_am, use_bqk, use_bfull, use_ln)
    nc = _get_program(flags)

    am_all = (1.0 - mask) * -10000.0  # [B, S]
    genvec = np.ascontiguousarray(
        np.stack([bfull_g, bfull_t, lnw_g, lnb_g, lnw_t, lnb_t]))

    in_maps = []
    for b in range(B):
        m = {
            "gt": np.ascontiguousarray(G[b].T),
            "tt": np.ascontiguousarray(T[b].T),
            "gn": np.ascontiguousarray(G[b]),
            "tn": np.ascontiguousarray(T[b]),
            "wq_g": wq_g, "wk_g": wk_g, "wv_g": wv_g,
            "wq_t": wq_t, "wk_t": wk_t, "wv_t": wv_t,
            "wo_g": wo_g, "wo_t": wo_t,
        }
        if use_am or use_bqk:
            consts = np.zeros((128, 72), f32)
            consts[:, 0:8] = am_all[b].reshape(8, 128).T
            consts[:, 8:24] = bq_eg.reshape(16, 128).T
            consts[:, 24:40] = bk_eg.reshape(16, 128).T
            consts[:, 40:56] = bq_et.reshape(16, 128).T
            consts[:, 56:72] = bk_et.reshape(16, 128).T
            m["consts"] = consts
        if use_bfull or use_ln:
            m["genvec"] = genvec
        in_maps.append(m)
    return nc, in_maps


def kernel(**inputs):
    nc, in_maps = prepare(**inputs)
    res = run_bass_kernel_spmd(nc, in_maps, core_ids=list(range(B)))
    H_G = np.stack([res.results[b]["hg"] for b in range(B)])
    H_T = np.stack([res.results[b]["ht"] for b in range(B)])
    return (H_G, H_T)
